# revision 13
# baseline (speedup 1.0000x reference)
"""CfC head (mLSTM-style scan) Trainium2 kernel, v2.

Math (per timestep t, per (b,h)):
    pre_g = xt*Wg_w + Wg_b            (xt = (x_codes-65)/100)
    i_t = exp(pre_i - n), f_t = exp(pre_f - n), o_t = exp(pre_o - n)
    g_t = sigmoid(pre_g); lam = sigmoid(pre_l)
    c   = f_t*c + i_t*g_t
    h   = (h + DT*o_t*sigmoid(c)) / (1 + DT*lam)
    n  += 0.01*(i_t + f_t + o_t - 3)
    y_t = h @ proj_w.T + proj_b

Device mapping: H=1024 sharded over 8 cores (128 h-values per core, one SBUF
partition each); free dim packs (batch-major, time-minor) blocks of TB steps.

n-recurrence: instead of a per-step drift scan, n is held constant within a
block at the mid-block value.  Per block, Se = sum_t (Ei+Ef+Eo) (one DVE
reduce); with SP = Se*exp(-Nc) the self-consistent block update is
    dn = (0.01*SP - 0.03*TB) / (1 + 0.005*SP)
(the denominator linearizes the within-block feedback of n on the gates), and
the gates are scaled by EN = exp(-(Nc + dn/2)) (mid-block centering).
Validated vs reference in fp16-emulating numpy: rel err 1.4e-3 at TB=64
(budget 2e-2); ablations: no-midpoint 5.3e-3, no-selfconsistency 6.6e-3.

c and h are exact affine scans given EN:
    c_t = (Ef_t*EN) * c_{t-1} + (Ei_t*G_t*EN)
    h_t = L1_t * h_{t-1} + L1D_t*Eo_t*EN*(Tc_t+1),  L1 = 1/(1+DT*lam)
L1 uses the Neumann form 1 - q + q^2 = (q-0.5)^2 + 0.75 (q = DT*lam <= 0.01),
fp32 (its error is amplified ~200x as the h-scan decay rate).  L1D = DT/2*L1
uses the first-order form DT/2*(1-q), affine in Tl = tanh(pre_l/2), so it is
one fp16 tensor_scalar (the dropped q^2 term is <=1e-4 relative on the
additive b-term; validated).  Sigmoids use tanh so every activation
(exp/tanh/square/identity) stays in the single "exp_and_others" ACT table.

Projection: pj [128,2] is the stationary matmul operand; h streams as rhs in
512-column chunks (one PSUM bank each), out [2, B*TB] per block.  Partials
over the 8 cores are summed on the host.

Emission is software-pipelined: block k+1's gate ACTs are emitted on ScalarE
before block k's Tc, and block k+1's gate-dependent DVE head (G, L1D, EiG,
Esum, reduce, dn-chain) fills the DVE bubble while ScalarE computes Tc(k).
"""

import os
from contextlib import ExitStack

import numpy as np

import concourse.bacc as bacc
import concourse.mybir as mybir
import concourse.tile as tile
from concourse.bass_utils import run_bass_kernel_spmd

AF = mybir.ActivationFunctionType
OP = mybir.AluOpType
F32 = mybir.dt.float32
F16 = mybir.dt.float16

B, S, H = 64, 2048, 1024
NCORES = 8
HC = H // NCORES  # 128 h-values per core = partition dim
DT = 0.01

TB = int(os.environ.get("KERNEL_TB", "64"))  # timesteps per block
# GpSimd offload bitmask: 1 = SqL1 +0.75, 2 = ENcF broadcast copy, 4 = Esum adds
GPS = int(os.environ.get("KERNEL_GPS", "7"))
CCLAMP = 3.0e4  # c-carry clamp; sigmoid(c>=17) == 1.0f so this is exact

_cached = {}
_last_results = None


def build_program(s=S, tb=TB):
    nb = s // tb
    nfd = B * tb           # free dim of block tiles, (b-major, t-minor)
    mmc = 512              # matmul chunk: [2, 512] fp32 out = one PSUM bank
    nmm = nfd // mmc

    nc = bacc.Bacc(
        "TRN2", target_bir_lowering=False, debug=False, num_devices=NCORES
    )
    # x pre-broadcast on the host to [nb, 128, B, tb]: each block's slab is one
    # contiguous 1 MB read (a 128-way partition_broadcast DMA measured ~9.9us
    # and serialized the gate ACTs behind it; this form is ~3x faster).
    x_d = nc.dram_tensor("x", [nb, 128, B, tb], F16, kind="ExternalInput").ap()
    wv_d = nc.dram_tensor("wv", [HC, 10], F32, kind="ExternalInput").ap()
    pj_d = nc.dram_tensor("projT", [HC, 2], F32, kind="ExternalInput").ap()
    n0_d = nc.dram_tensor("n0", [HC, 1], F32, kind="ExternalInput").ap()
    y_d = nc.dram_tensor("yout", [nb, 2, nfd], F32, kind="ExternalOutput").ap()

    def r3(ap):  # [128, nfd] -> [128, B, tb]
        return ap.rearrange("p (b t) -> p b t", t=tb)

    with tile.TileContext(nc) as tc, ExitStack() as ctx:
        wp = ctx.enter_context(tc.tile_pool(name="w", bufs=1))
        pha = ctx.enter_context(tc.tile_pool(name="pha", bufs=2))
        chn = ctx.enter_context(tc.tile_pool(name="chn", bufs=1))
        pp = ctx.enter_context(tc.tile_pool(name="pp", bufs=1, space="PSUM"))
        smp = ctx.enter_context(tc.tile_pool(name="smp", bufs=1))

        wv = wp.tile([HC, 10], F32)
        nc.sync.dma_start(wv[:], wv_d)
        pj = wp.tile([HC, 2], F32)
        nc.sync.dma_start(pj[:], pj_d)
        n0t = wp.tile([HC, 1], F32)
        nc.sync.dma_start(n0t[:], n0_d)

        # persistent state and per-block scratch (one buffer each)
        Nc = wp.tile([HC, B], F32)
        nc.vector.memset(Nc[:], 0.0)
        nc.vector.tensor_scalar(Nc[:], Nc[:], n0t[:, 0:1], None, OP.add)
        ENc0 = wp.tile([HC, B], F16)   # exp(-Nc) at block start
        nc.scalar.activation(ENc0[:], Nc[:], AF.Exp, scale=-1.0)
        ENc = wp.tile([HC, B], F16)    # exp(-(Nc + dn/2)) mid-block
        Ccl = wp.tile([HC, B], F16)    # clamped c carry
        nc.vector.memset(Ccl[:], 0.0)
        hz = wp.tile([HC, B], F32)     # zero h carry for block 0
        nc.vector.memset(hz[:], 0.0)
        bqm = wp.tile([HC, 1], F32)
        nc.vector.memset(bqm[:], DT / 2 - 0.5)
        b75 = wp.tile([HC, 1], F32)
        nc.vector.memset(b75[:], 0.75)
        Se = wp.tile([HC, B], F32)
        SPt = wp.tile([HC, B], F32)
        numt = wp.tile([HC, B], F32)
        dent = wp.tile([HC, B], F32)
        rdent = wp.tile([HC, B], F32)
        dnt = wp.tile([HC, B], F32)
        Nargt = wp.tile([HC, B], F32)
        t64 = wp.tile([HC, B], F16)
        t64b = wp.tile([HC, B], F32)

        # block-cycled tiles (single buffer; in-order engines keep them safe)
        ENcF = chn.tile([HC, nfd], F16, tag="ENcF")
        ct = chn.tile([HC, nfd], F16, tag="c")
        Tc = chn.tile([HC, nfd], F16, tag="Tc")
        L1D = chn.tile([HC, nfd], F16, tag="L1D")
        ht = chn.tile([HC, nfd], F32, tag="h")
        ps = pp.tile([2, nfd], F32)
        ysb = smp.tile([2, nfd], F32)

        def prep_sc(k):
            """DMA + gate ACTs for block k (ScalarE stream; tanh first so the
            DVE head can start before the exps finish)."""
            d = {}
            d["X"] = pha.tile([128, nfd], F16, tag="X", name="X", bufs=3)
            nc.sync.dma_start(r3(d["X"][:]), x_d[k])
            d["Tg"] = pha.tile([128, nfd], F16, tag="Tg", name="Tg")
            nc.scalar.activation(
                d["Tg"][:], d["X"][:], AF.Tanh, bias=wv[:, 7:8], scale=wv[:, 6:7]
            )
            d["Tl"] = pha.tile([128, nfd], F16, tag="Tl", name="Tl")
            nc.scalar.activation(
                d["Tl"][:], d["X"][:], AF.Tanh, bias=wv[:, 9:10], scale=wv[:, 8:9]
            )
            d["Ei"] = pha.tile([128, nfd], F16, tag="Ei", name="Ei")
            nc.scalar.activation(
                d["Ei"][:], d["X"][:], AF.Exp, bias=wv[:, 1:2], scale=wv[:, 0:1]
            )
            d["Ef"] = pha.tile([128, nfd], F16, tag="Ef", name="Ef")
            nc.scalar.activation(
                d["Ef"][:], d["X"][:], AF.Exp, bias=wv[:, 3:4], scale=wv[:, 2:3]
            )
            d["Eo"] = pha.tile([128, nfd], F16, tag="Eo", name="Eo")
            nc.scalar.activation(
                d["Eo"][:], d["X"][:], AF.Exp, bias=wv[:, 5:6], scale=wv[:, 4:5]
            )
            return d

        def prep_sq_sc(d):
            # SqL1 = (DT/2*Tl + (DT/2-0.5))^2 + 0.75 = 1 - q + q^2, fp32
            d["SqL1"] = pha.tile([128, nfd], F32, tag="SqL1", name="SqL1")
            nc.scalar.activation(
                d["SqL1"][:], d["Tl"][:], AF.Square, bias=bqm[:], scale=DT / 2
            )
            if GPS & 1:
                nc.gpsimd.tensor_scalar(
                    d["SqL1"][:], d["SqL1"][:], 0.75, None, OP.add
                )
            else:
                nc.scalar.activation(
                    d["SqL1"][:], d["SqL1"][:], AF.Identity, bias=b75[:]
                )

        def prep_dve(d):
            """Gate-dependent DVE head: G, L1D, EiG, Esum, reduce, dn chain."""
            # G = 0.5*Tg+0.5 ; EiG = Ei*G (lands in Tg)
            nc.vector.tensor_scalar(d["Tg"][:], d["Tg"][:], 0.5, 0.5, OP.mult, OP.add)
            # L1D = DT/2*(1 - q) = -DT^2/4 * Tl + (DT/2 - DT^2/4)
            nc.vector.tensor_scalar(
                L1D[:], d["Tl"][:], -DT * DT / 4, DT / 2 - DT * DT / 4,
                OP.mult, OP.add,
            )
            nc.vector.tensor_mul(d["Tg"][:], d["Ei"][:], d["Tg"][:])
            # Esum = Ei+Ef+Eo (lands in Ei), then u = Eo*L1D (lands in Eo)
            eng = nc.gpsimd if GPS & 4 else nc.vector
            eng.tensor_add(d["Ei"][:], d["Ei"][:], d["Ef"][:])
            eng.tensor_add(d["Ei"][:], d["Ei"][:], d["Eo"][:])
            nc.vector.tensor_mul(d["Eo"][:], d["Eo"][:], L1D[:])
            nc.vector.tensor_reduce(
                Se[:], r3(d["Ei"][:]), axis=mybir.AxisListType.X, op=OP.add
            )
            # dn = (0.01*SP - 0.03*tb)/(1 + 0.005*SP), SP = Se*exp(-Nc)
            nc.vector.tensor_mul(SPt[:], Se[:], ENc0[:])
            nc.vector.tensor_scalar(
                numt[:], SPt[:], 0.01, -0.03 * tb, OP.mult, OP.add
            )
            nc.vector.tensor_scalar(dent[:], SPt[:], 0.005, 1.0, OP.mult, OP.add)
            nc.vector.reciprocal(rdent[:], dent[:])
            nc.vector.tensor_mul(dnt[:], numt[:], rdent[:])
            nc.vector.scalar_tensor_tensor(
                Nargt[:], dnt[:], 0.5, Nc[:], OP.mult, OP.add
            )
            nc.vector.tensor_add(Nc[:], Nc[:], dnt[:])

        def prep_en_sc():
            nc.scalar.activation(ENc[:], Nargt[:], AF.Exp, scale=-1.0)
            nc.scalar.activation(ENc0[:], Nc[:], AF.Exp, scale=-1.0)

        def prep_encf():
            eng = nc.gpsimd if GPS & 2 else nc.vector
            eng.tensor_copy(
                r3(ENcF[:]), ENc[:].unsqueeze(2).broadcast_to([HC, B, tb])
            )

        # ---- prologue: full prep of block 0
        cur = prep_sc(0)
        prep_sq_sc(cur)
        prep_dve(cur)
        prep_en_sc()
        prep_encf()

        for k in range(nb):
            last = k == nb - 1
            if not last:
                nxt = prep_sc(k + 1)    # ScalarE: gates(k+1) before Tc(k)

            # c-scan coefficients: a_c = Ef*EN (in Ef), b_c = EiG*EN (in Tg)
            nc.vector.tensor_mul(cur["Ef"][:], cur["Ef"][:], ENcF[:])
            nc.vector.tensor_mul(cur["Tg"][:], cur["Tg"][:], ENcF[:])
            nc.vector.tensor_mul(t64[:], r3(cur["Ef"][:])[:, :, 0], Ccl[:])
            nc.vector.tensor_add(
                r3(cur["Tg"][:])[:, :, 0], r3(cur["Tg"][:])[:, :, 0], t64[:]
            )
            nc.vector.memset(r3(cur["Ef"][:])[:, :, 0], 0.0)
            nc.vector.tensor_tensor_scan(
                ct[:], cur["Ef"][:], cur["Tg"][:], 0.0, OP.mult, OP.add
            )

            nc.scalar.activation(Tc[:], ct[:], AF.Tanh, scale=0.5)

            if not last:                # DVE bubble-fill while ScalarE does Tc
                prep_sq_sc(nxt)
                prep_dve(nxt)
                prep_en_sc()

            # b_h = u*(Tc+1)*EN  (u = Eo*L1D, already in Eo)
            nc.vector.tensor_scalar(Tc[:], Tc[:], 1.0, None, OP.add)
            nc.vector.tensor_mul(cur["Eo"][:], cur["Eo"][:], Tc[:])
            nc.vector.tensor_mul(cur["Eo"][:], cur["Eo"][:], ENcF[:])
            if not last:
                prep_encf()             # EN broadcast for block k+1
            hprev = hz[:] if k == 0 else r3(ht[:])[:, :, tb - 1]
            nc.vector.tensor_mul(t64b[:], r3(cur["SqL1"][:])[:, :, 0], hprev)
            nc.vector.tensor_add(
                r3(cur["Eo"][:])[:, :, 0], r3(cur["Eo"][:])[:, :, 0], t64b[:]
            )
            nc.vector.memset(r3(cur["SqL1"][:])[:, :, 0], 0.0)
            nc.vector.tensor_tensor_scan(
                ht[:], cur["SqL1"][:], cur["Eo"][:], 0.0, OP.mult, OP.add
            )
            nc.vector.tensor_scalar_min(
                Ccl[:], r3(ct[:])[:, :, tb - 1], CCLAMP
            )

            # y partials: ps[2, nfd] = pj.T @ h, in one-PSUM-bank chunks
            for j in range(nmm):
                nc.tensor.matmul(
                    ps[:, j * mmc : (j + 1) * mmc],
                    pj[:],
                    ht[:, j * mmc : (j + 1) * mmc],
                    start=True,
                    stop=True,
                )
            nc.scalar.copy(ysb[:], ps[:])
            nc.sync.dma_start(y_d[k], ysb[:])

            if not last:
                cur = nxt

    nc.compile()
    return nc


def _get_program():
    key = (S, TB)
    if key not in _cached:
        _cached[key] = build_program(S, TB)
    return _cached[key]


def host_inputs(x_codes, Wi_w, Wi_b, Wf_w, Wf_b, Wo_w, Wo_b, Wg_w, Wg_b,
                Wl_w, Wl_b, proj_w, proj_b, n_init):
    """Fold input normalization into per-gate ACT scale/bias; shard over H."""
    f = lambda v: np.asarray(v, np.float32)
    cols = []
    for (w, b) in ((Wi_w, Wi_b), (Wf_w, Wf_b), (Wo_w, Wo_b)):
        cols += [f(w) / 100.0, f(b) - 0.65 * f(w)]
    for (w, b) in ((Wg_w, Wg_b), (Wl_w, Wl_b)):
        cols += [f(w) / 200.0, (f(b) - 0.65 * f(w)) / 2.0]
    wv_full = np.stack(cols, axis=1).astype(np.float32)  # [H, 10]
    nb = S // TB
    xr = f(x_codes).astype(np.float16).reshape(B, nb, TB).transpose(1, 0, 2)
    x = np.ascontiguousarray(
        np.broadcast_to(xr[:, None], (nb, 128, B, TB))
    )  # [nb, 128, B, TB], each block one contiguous slab
    pw = f(proj_w)
    n0 = f(n_init)
    maps = []
    for k in range(NCORES):
        hs = slice(k * HC, (k + 1) * HC)
        maps.append({
            "x": x,
            "wv": np.ascontiguousarray(wv_full[hs]),
            "projT": np.ascontiguousarray(pw[:, hs].T),
            "n0": np.ascontiguousarray(n0[hs].reshape(HC, 1)),
        })
    return maps


def assemble_output(results, proj_b, s=S, tb=TB):
    nb = s // tb
    y = np.zeros((B, s, 2), np.float64)
    for k in range(NCORES):
        yc = np.asarray(results[k]["yout"], np.float64)  # [nb, 2, B*tb]
        ycr = yc.reshape(nb, 2, B, tb)
        y += np.transpose(ycr, (2, 0, 3, 1)).reshape(B, s, 2)
    y += np.asarray(proj_b, np.float64)[None, None, :]
    return y.astype(np.float32)


def kernel(**inputs):
    global _last_results
    nc = _get_program()
    maps = host_inputs(**inputs)
    res = run_bass_kernel_spmd(
        nc, maps, list(range(NCORES)),
        trace=bool(os.environ.get("KTRACE")),
        tmpdir=os.environ.get("KTRACE_DIR") or None,
    )
    _last_results = res
    return assemble_output(res.results, inputs["proj_b"])


# revision 19
# speedup vs baseline: 1.2899x; 1.2899x over previous
"""CfC head (mLSTM-style scan) Trainium2 kernel, v2.

Math (per timestep t, per (b,h)):
    pre_g = xt*Wg_w + Wg_b            (xt = (x_codes-65)/100)
    i_t = exp(pre_i - n), f_t = exp(pre_f - n), o_t = exp(pre_o - n)
    g_t = sigmoid(pre_g); lam = sigmoid(pre_l)
    c   = f_t*c + i_t*g_t
    h   = (h + DT*o_t*sigmoid(c)) / (1 + DT*lam)
    n  += 0.01*(i_t + f_t + o_t - 3)
    y_t = h @ proj_w.T + proj_b

Device mapping: H=1024 sharded over 8 cores (128 h-values per core, one SBUF
partition each); free dim packs (batch-major, time-minor) blocks of TB steps.

n-recurrence: instead of a per-step drift scan, n is held constant within a
block at the mid-block value.  Per block, Se = sum_t (Ei+Ef+Eo) (one DVE
reduce); with SP = Se*exp(-Nc) the self-consistent block update is
    dn = (0.01*SP - 0.03*TB) / (1 + 0.005*SP)
(the denominator linearizes the within-block feedback of n on the gates), and
the gates are scaled by EN = exp(-(Nc + dn/2)) (mid-block centering).
Validated vs reference in fp16-emulating numpy: rel err 1.4e-3 at TB=64
(budget 2e-2); ablations: no-midpoint 5.3e-3, no-selfconsistency 6.6e-3.

c and h are exact affine scans given EN:
    c_t = (Ef_t*EN) * c_{t-1} + (Ei_t*G_t*EN)
    h_t = L1_t * h_{t-1} + L1D_t*Eo_t*EN*(Tc_t+1),  L1 = 1/(1+DT*lam)
L1 uses the Neumann form 1 - q + q^2 = (q-0.5)^2 + 0.75 (q = DT*lam <= 0.01),
fp32 (its error is amplified ~200x as the h-scan decay rate).  L1D = DT/2*L1
uses the first-order form DT/2*(1-q), affine in Tl = tanh(pre_l/2), so it is
one fp16 tensor_scalar (the dropped q^2 term is <=1e-4 relative on the
additive b-term; validated).  Sigmoids use tanh so every activation
(exp/tanh/square/identity) stays in the single "exp_and_others" ACT table.

Projection: pj [128,2] is the stationary matmul operand; h streams as rhs in
512-column chunks (one PSUM bank each), out [2, B*TB] per block.  Partials
over the 8 cores are summed on the host.

Emission is software-pipelined: block k+1's gate ACTs are emitted on ScalarE
before block k's Tc, and block k+1's gate-dependent DVE head (G, L1D, EiG,
Esum, reduce, dn-chain) fills the DVE bubble while ScalarE computes Tc(k).
"""

import os
from contextlib import ExitStack

import numpy as np

import concourse.bacc as bacc
import concourse.mybir as mybir
import concourse.tile as tile
from concourse.bass_utils import run_bass_kernel_spmd

AF = mybir.ActivationFunctionType
OP = mybir.AluOpType
F32 = mybir.dt.float32
F16 = mybir.dt.float16

B, S, H = 64, 2048, 1024
NCORES = 8
HC = H // NCORES  # 128 h-values per core = partition dim
DT = 0.01

TB = int(os.environ.get("KERNEL_TB", "64"))  # timesteps per block
# GpSimd offload bitmask: 1 = SqL1 +0.75, 2 = ENcF broadcast copy, 4 = Esum adds
GPS = int(os.environ.get("KERNEL_GPS", "7"))
CCLAMP = 3.0e4  # c-carry clamp; sigmoid(c>=17) == 1.0f so this is exact

_cached = {}
_last_results = None


def build_program(s=S, tb=TB):
    nb = s // tb
    nfd = B * tb           # free dim of block tiles, (b-major, t-minor)
    mmc = 512              # matmul chunk: [2, 512] fp32 out = one PSUM bank
    nmm = nfd // mmc

    nc = bacc.Bacc(
        "TRN2", target_bir_lowering=False, debug=False, num_devices=NCORES
    )
    # x pre-broadcast on the host to [nb, 128, B, tb]: each block's slab is one
    # contiguous 1 MB read (a 128-way partition_broadcast DMA measured ~9.9us
    # and serialized the gate ACTs behind it; this form is ~3x faster).
    x_d = nc.dram_tensor("x", [nb, 128, B, tb], F16, kind="ExternalInput").ap()
    wv_d = nc.dram_tensor("wv", [HC, 10], F32, kind="ExternalInput").ap()
    pj_d = nc.dram_tensor("projT", [HC, 2], F32, kind="ExternalInput").ap()
    n0_d = nc.dram_tensor("n0", [HC, 1], F32, kind="ExternalInput").ap()
    y_d = nc.dram_tensor("yout", [nb, 2, nfd], F32, kind="ExternalOutput").ap()

    def r3(ap):  # [128, nfd] -> [128, B, tb]
        return ap.rearrange("p (b t) -> p b t", t=tb)

    with tile.TileContext(nc) as tc, ExitStack() as ctx:
        wp = ctx.enter_context(tc.tile_pool(name="w", bufs=1))
        pha = ctx.enter_context(tc.tile_pool(name="pha", bufs=2))
        chn = ctx.enter_context(tc.tile_pool(name="chn", bufs=1))
        pp = ctx.enter_context(tc.tile_pool(name="pp", bufs=1, space="PSUM"))
        smp = ctx.enter_context(tc.tile_pool(name="smp", bufs=1))

        wv = wp.tile([HC, 10], F32)
        nc.sync.dma_start(wv[:], wv_d)
        pj = wp.tile([HC, 2], F32)
        nc.sync.dma_start(pj[:], pj_d)
        n0t = wp.tile([HC, 1], F32)
        nc.sync.dma_start(n0t[:], n0_d)

        # persistent state and per-block scratch (one buffer each)
        Nc = wp.tile([HC, B], F32)
        nc.vector.memset(Nc[:], 0.0)
        nc.vector.tensor_scalar(Nc[:], Nc[:], n0t[:, 0:1], None, OP.add)
        ENc0 = wp.tile([HC, B], F16)   # exp(-Nc) at block start
        nc.scalar.activation(ENc0[:], Nc[:], AF.Exp, scale=-1.0)
        ENc = wp.tile([HC, B], F16)    # exp(-(Nc + dn/2)) mid-block
        Ccl = wp.tile([HC, B], F16)    # clamped c carry
        nc.vector.memset(Ccl[:], 0.0)
        hz = wp.tile([HC, B], F32)     # zero h carry for block 0
        nc.vector.memset(hz[:], 0.0)
        bqm = wp.tile([HC, 1], F32)
        nc.vector.memset(bqm[:], DT / 2 - 0.5)
        b75 = wp.tile([HC, 1], F32)
        nc.vector.memset(b75[:], 0.75)
        Se = wp.tile([HC, B], F32)
        SPt = wp.tile([HC, B], F32)
        numt = wp.tile([HC, B], F32)
        dent = wp.tile([HC, B], F32)
        rdent = wp.tile([HC, B], F32)
        dnt = wp.tile([HC, B], F32)
        Nargt = wp.tile([HC, B], F32)
        t64 = wp.tile([HC, B], F16)
        t64b = wp.tile([HC, B], F32)

        # block-cycled tiles (single buffer; in-order engines keep them safe)
        ENcF = chn.tile([HC, nfd], F16, tag="ENcF")
        ct = chn.tile([HC, nfd], F16, tag="c")
        Tc = chn.tile([HC, nfd], F16, tag="Tc")
        ht = chn.tile([HC, nfd], F32, tag="h")
        ps = pp.tile([2, nfd], F32)
        ysb = smp.tile([2, nfd], F32)

        def prep_sc(k):
            """DMA + gate ACTs for block k (ScalarE stream; tanh first so the
            DVE head can start before the exps finish)."""
            d = {}
            d["X"] = pha.tile([128, nfd], F16, tag="X", name="X", bufs=3)
            nc.sync.dma_start(r3(d["X"][:]), x_d[k])
            d["Tg"] = pha.tile([128, nfd], F16, tag="Tg", name="Tg")
            nc.scalar.activation(
                d["Tg"][:], d["X"][:], AF.Tanh, bias=wv[:, 7:8], scale=wv[:, 6:7]
            )
            d["Tl"] = pha.tile([128, nfd], F16, tag="Tl", name="Tl")
            nc.scalar.activation(
                d["Tl"][:], d["X"][:], AF.Tanh, bias=wv[:, 9:10], scale=wv[:, 8:9]
            )
            d["Ei"] = pha.tile([128, nfd], F16, tag="Ei", name="Ei")
            nc.scalar.activation(
                d["Ei"][:], d["X"][:], AF.Exp, bias=wv[:, 1:2], scale=wv[:, 0:1]
            )
            d["Ef"] = pha.tile([128, nfd], F16, tag="Ef", name="Ef")
            nc.scalar.activation(
                d["Ef"][:], d["X"][:], AF.Exp, bias=wv[:, 3:4], scale=wv[:, 2:3]
            )
            d["Eo"] = pha.tile([128, nfd], F16, tag="Eo", name="Eo")
            nc.scalar.activation(
                d["Eo"][:], d["X"][:], AF.Exp, bias=wv[:, 5:6], scale=wv[:, 4:5]
            )
            return d

        def prep_sq_sc(d):
            # SqL1 = (DT/2*Tl + (DT/2-0.5))^2 + 0.75 = 1 - q + q^2, fp32
            d["SqL1"] = pha.tile([128, nfd], F32, tag="SqL1", name="SqL1")
            nc.scalar.activation(
                d["SqL1"][:], d["Tl"][:], AF.Square, bias=bqm[:], scale=DT / 2
            )
            if GPS & 1:
                nc.gpsimd.tensor_scalar(
                    d["SqL1"][:], d["SqL1"][:], 0.75, None, OP.add
                )
            else:
                nc.scalar.activation(
                    d["SqL1"][:], d["SqL1"][:], AF.Identity, bias=b75[:]
                )

        def prep_dve(d):
            """Gate-dependent DVE head: G, L1D, EiG, Esum, reduce, dn chain."""
            # G = 0.5*Tg+0.5 ; EiG = Ei*G (lands in Tg)
            nc.vector.tensor_scalar(d["Tg"][:], d["Tg"][:], 0.5, 0.5, OP.mult, OP.add)
            nc.vector.tensor_mul(d["Tg"][:], d["Ei"][:], d["Tg"][:])
            # Esum = Ei+Ef+Eo (lands in Ei)
            eng = nc.gpsimd if GPS & 4 else nc.vector
            eng.tensor_add(d["Ei"][:], d["Ei"][:], d["Ef"][:])
            eng.tensor_add(d["Ei"][:], d["Ei"][:], d["Eo"][:])
            nc.vector.tensor_reduce(
                Se[:], r3(d["Ei"][:]), axis=mybir.AxisListType.X, op=OP.add
            )
            # dn = (0.01*SP - 0.03*tb)/(1 + 0.005*SP), SP = Se*exp(-Nc)
            nc.vector.tensor_mul(SPt[:], Se[:], ENc0[:])
            nc.vector.tensor_scalar(
                numt[:], SPt[:], 0.01, -0.03 * tb, OP.mult, OP.add
            )
            nc.vector.tensor_scalar(dent[:], SPt[:], 0.005, 1.0, OP.mult, OP.add)
            nc.vector.reciprocal(rdent[:], dent[:])
            nc.vector.tensor_mul(dnt[:], numt[:], rdent[:])
            nc.vector.scalar_tensor_tensor(
                Nargt[:], dnt[:], 0.5, Nc[:], OP.mult, OP.add
            )
            nc.vector.tensor_add(Nc[:], Nc[:], dnt[:])

        def prep_en_sc():
            nc.scalar.activation(ENc[:], Nargt[:], AF.Exp, scale=-1.0)
            nc.scalar.activation(ENc0[:], Nc[:], AF.Exp, scale=-1.0)

        def prep_encf():
            # broadcast EN over t on ScalarE (ACT Copy reads the stride-0 view);
            # keeps the DVE free for the scan-critical chain
            nc.scalar.activation(
                r3(ENcF[:]),
                ENc[:].unsqueeze(2).broadcast_to([HC, B, tb]),
                AF.Copy,
            )

        # ---- prologue: full prep of block 0
        cur = prep_sc(0)
        prep_sq_sc(cur)
        prep_dve(cur)
        prep_en_sc()
        prep_encf()

        for k in range(nb):
            last = k == nb - 1
            if not last:
                nxt = prep_sc(k + 1)    # ScalarE: gates(k+1) before Tc(k)

            # c-scan coefficients: a_c = Ef*EN (in Ef), b_c = EiG*EN (in Tg)
            nc.vector.tensor_mul(cur["Ef"][:], cur["Ef"][:], ENcF[:])
            nc.vector.tensor_mul(cur["Tg"][:], cur["Tg"][:], ENcF[:])
            nc.vector.tensor_mul(t64[:], r3(cur["Ef"][:])[:, :, 0], Ccl[:])
            nc.vector.tensor_add(
                r3(cur["Tg"][:])[:, :, 0], r3(cur["Tg"][:])[:, :, 0], t64[:]
            )
            nc.vector.memset(r3(cur["Ef"][:])[:, :, 0], 0.0)
            nc.vector.tensor_tensor_scan(
                ct[:], cur["Ef"][:], cur["Tg"][:], 0.0, OP.mult, OP.add
            )

            nc.scalar.activation(Tc[:], ct[:], AF.Tanh, scale=0.5)

            if not last:                # DVE bubble-fill while ScalarE does Tc
                prep_sq_sc(nxt)
                prep_dve(nxt)
                prep_en_sc()

            # b_h = Eo*(Tc+1)*EN; the DT/2*(1-q) factor is folded into projT
            # on the host (per-lane mean-q correction; validated rel 1.5e-3)
            nc.vector.tensor_scalar(Tc[:], Tc[:], 1.0, None, OP.add)
            nc.vector.tensor_mul(cur["Eo"][:], cur["Eo"][:], Tc[:])
            nc.vector.tensor_mul(cur["Eo"][:], cur["Eo"][:], ENcF[:])
            if not last:
                prep_encf()             # EN broadcast for block k+1
            hprev = hz[:] if k == 0 else r3(ht[:])[:, :, tb - 1]
            nc.vector.tensor_mul(t64b[:], r3(cur["SqL1"][:])[:, :, 0], hprev)
            nc.vector.tensor_add(
                r3(cur["Eo"][:])[:, :, 0], r3(cur["Eo"][:])[:, :, 0], t64b[:]
            )
            nc.vector.memset(r3(cur["SqL1"][:])[:, :, 0], 0.0)
            nc.vector.tensor_tensor_scan(
                ht[:], cur["SqL1"][:], cur["Eo"][:], 0.0, OP.mult, OP.add
            )
            nc.vector.tensor_scalar_min(
                Ccl[:], r3(ct[:])[:, :, tb - 1], CCLAMP
            )

            # y partials: ps[2, nfd] = pj.T @ h, in one-PSUM-bank chunks
            for j in range(nmm):
                nc.tensor.matmul(
                    ps[:, j * mmc : (j + 1) * mmc],
                    pj[:],
                    ht[:, j * mmc : (j + 1) * mmc],
                    start=True,
                    stop=True,
                )
            nc.scalar.copy(ysb[:], ps[:])
            nc.sync.dma_start(y_d[k], ysb[:])

            if not last:
                cur = nxt

    nc.compile()
    return nc


def _get_program():
    key = (S, TB)
    if key not in _cached:
        _cached[key] = build_program(S, TB)
    return _cached[key]


def host_inputs(x_codes, Wi_w, Wi_b, Wf_w, Wf_b, Wo_w, Wo_b, Wg_w, Wg_b,
                Wl_w, Wl_b, proj_w, proj_b, n_init):
    """Fold input normalization into per-gate ACT scale/bias; shard over H."""
    f = lambda v: np.asarray(v, np.float32)
    cols = []
    for (w, b) in ((Wi_w, Wi_b), (Wf_w, Wf_b), (Wo_w, Wo_b)):
        cols += [f(w) / 100.0, f(b) - 0.65 * f(w)]
    for (w, b) in ((Wg_w, Wg_b), (Wl_w, Wl_b)):
        cols += [f(w) / 200.0, (f(b) - 0.65 * f(w)) / 2.0]
    wv_full = np.stack(cols, axis=1).astype(np.float32)  # [H, 10]
    nb = S // TB
    xr = f(x_codes).astype(np.float16).reshape(B, nb, TB).transpose(1, 0, 2)
    x = np.ascontiguousarray(
        np.broadcast_to(xr[:, None], (nb, 128, B, TB))
    )  # [nb, 128, B, TB], each block one contiguous slab
    # fold DT/2 * (1 - DT*E[sigmoid(pre_l)]) into the projection (per-lane
    # mean-q correction for dropping the per-element (1-q) factor from b_h;
    # probit approximation of the mean over x ~ N(0, 0.1))
    wl, bl = f(Wl_w), f(Wl_b)
    sigbar = 1.0 / (1.0 + np.exp(
        -bl / np.sqrt(1.0 + np.pi * (0.1 * wl) ** 2 / 8.0)
    ))
    pw = f(proj_w) * (DT / 2 * (1.0 - DT * sigbar))[None, :]
    n0 = f(n_init)
    maps = []
    for k in range(NCORES):
        hs = slice(k * HC, (k + 1) * HC)
        maps.append({
            "x": x,
            "wv": np.ascontiguousarray(wv_full[hs]),
            "projT": np.ascontiguousarray(pw[:, hs].T),
            "n0": np.ascontiguousarray(n0[hs].reshape(HC, 1)),
        })
    return maps


def assemble_output(results, proj_b, s=S, tb=TB):
    nb = s // tb
    y = np.zeros((B, s, 2), np.float64)
    for k in range(NCORES):
        yc = np.asarray(results[k]["yout"], np.float64)  # [nb, 2, B*tb]
        ycr = yc.reshape(nb, 2, B, tb)
        y += np.transpose(ycr, (2, 0, 3, 1)).reshape(B, s, 2)
    y += np.asarray(proj_b, np.float64)[None, None, :]
    return y.astype(np.float32)


def kernel(**inputs):
    global _last_results
    nc = _get_program()
    maps = host_inputs(**inputs)
    res = run_bass_kernel_spmd(
        nc, maps, list(range(NCORES)),
        trace=bool(os.environ.get("KTRACE")),
        tmpdir=os.environ.get("KTRACE_DIR") or None,
    )
    _last_results = res
    return assemble_output(res.results, inputs["proj_b"])
